# revision 21
# baseline (speedup 1.0000x reference)
"""Trainium2 Bass kernel for nn_CirculantSTRING.

Math: out[b,n,:] = irfft(exp(i*theta(n,:)) * rfft(x[b,n,:]), n=D)
where theta(n,f) = 2*(p0[n]*Im(rfft(circ0))[f] + p1[n]*Im(rfft(circ1))[f]).

Per core (data-parallel over batch, 4 batches/core), folded real-DFT:
  - even/odd fold: eo = [e_0..e_383 | x_384, o_1..o_383],
    e_d = x_d + x_{768-d}, o_d = x_d - x_{768-d} (halves forward matmul work)
  - forward matmul to fi = [R_0..R_383 | R_384, I_1..I_383]
    (block-sparse F2: 24 of 36 (128,128) blocks)
  - phase rotation with on-device cos/sin(theta) tables; theta(n,0)=0 makes
    the R_384 slot (chunk 3, partition 0) pass through untouched
  - inverse matmul to uv = [u_0..u_384 | v_1..v_383] (7 matmuls/row-group),
    un-fold out[d] = u_d - v_d, out[768-d] = u_d + v_d
Matmuls in float32r (1 cyc/row on PE at N>=256, ~11 mantissa bits); phase
path in fp32.
"""
import math
from contextlib import ExitStack

import numpy as np

import concourse.bacc as bacc
import concourse.tile as tile
from concourse import mybir
from concourse import bass_utils
from concourse.masks import make_identity

F32 = mybir.dt.float32
F32R = mybir.dt.float32r
I32 = mybir.dt.int32

B, N, D = 32, 1024, 768
NCORES = 8
BS = B // NCORES
P = 128
NCH = D // P              # 6
ROWTILE = 512
NG = ROWTILE // P         # 4

TWOPI = 2.0 * math.pi

# forward block list: M-chunk -> list of K-chunks
FWD_BLOCKS = {0: [0, 1, 2, 3], 1: [0, 1, 2, 3], 2: [0, 1, 2, 3],
              3: [0, 1, 2, 3, 4, 5], 4: [3, 4, 5], 5: [3, 4, 5]}


def _dft_matrices():
    """Symmetric base matrices: C (385,385) cos incl boundary row/col,
    S (384,384) sin (row/col 0 are zero)."""
    dc = np.arange(385)
    C = np.cos(2 * np.pi * np.outer(dc, dc) / D).astype(np.float32)
    ds_ = np.arange(384)
    S = np.sin(2 * np.pi * np.outer(ds_, ds_) / D).astype(np.float32)
    return C, S


def build_kernel(mm_dtype=F32R, reps=1, trace_sim=False):
    nc = bacc.Bacc("TRN2", target_bir_lowering=False, debug=False,
                   num_devices=NCORES)
    x = nc.dram_tensor("x", [BS, N, D], F32, kind="ExternalInput").ap()
    circ = nc.dram_tensor("circ", [2, D], F32, kind="ExternalInput").ap()
    positions = nc.dram_tensor("positions", [N, 2], I32,
                               kind="ExternalInput").ap()
    cs_c = nc.dram_tensor("cs_c", [385, 385], F32, kind="ExternalInput").ap()
    ss_c = nc.dram_tensor("ss_c", [384, 384], F32, kind="ExternalInput").ap()
    out = nc.dram_tensor("out", [BS, N, D], F32, kind="ExternalOutput").ap()

    with tile.TileContext(nc, trace_sim=trace_sim) as tc, ExitStack() as ctx:
        consts = ctx.enter_context(tc.tile_pool(name="consts", bufs=1))
        stage = ctx.enter_context(tc.tile_pool(name="stage", bufs=1))
        tabs = ctx.enter_context(tc.tile_pool(name="tabs", bufs=1))
        xio = ctx.enter_context(tc.tile_pool(name="xio", bufs=2))
        work = ctx.enter_context(tc.tile_pool(name="work", bufs=2))

        ident = consts.tile([P, P], F32, tag="ident")
        make_identity(nc, ident)

        ps0 = tc.tile_pool(name="ps0", bufs=1, space="PSUM")
        psum = ps0.__enter__()
        hp = tc.high_priority()
        hp.__enter__()

        # ---- circ odd-fold (for s2, in fp32) ----
        circR = tabs.tile([2, D], F32, tag="circR")
        nc.sync.dma_start(out=circR, in_=circ)
        ocr = tabs.tile([2, 384], F32, tag="ocr")
        nc.vector.memset(ocr[:, 0:1], 0.0)
        nc.vector.tensor_sub(ocr[:, 1:384], circR[:, 1:384],
                             circR[:, 767:384:-1])
        occ = []  # (128, 2) fp32, o-fold of circ on chunk 3..5 partitions
        for i in range(3):
            poc = psum.tile([P, 2], F32, tag="pocc")
            nc.tensor.transpose(poc, ocr[:, i * P:(i + 1) * P], ident[0:2, 0:2])
            so = tabs.tile([P, 2], F32, tag=f"occ{i}")
            nc.scalar.copy(out=so, in_=poc)
            occ.append(so)

        # ---- load C/S base matrices, assemble F2/G2 tiles, s2 matmul ----
        Cst, Sst = [], []
        for i in range(3):
            t_s = stage.tile([P, 384], F32, tag=f"sst{i}", name=f"sst{i}")
            nc.sync.dma_start(out=t_s, in_=ss_c[i * P:(i + 1) * P, :])
            Sst.append(t_s)
        for i in range(3):
            t_c = stage.tile([P, 385], F32, tag=f"cst{i}", name=f"cst{i}")
            nc.sync.dma_start(out=t_c, in_=cs_c[i * P:(i + 1) * P, :])
            Cst.append(t_c)
        c384 = stage.tile([1, 385], F32, tag="c384")
        nc.sync.dma_start(out=c384, in_=cs_c[384:385, :])

        # s2' = sum_i occ[i]^T @ S-chunk (theta sign absorbed into posTf)
        s2ps = psum.tile([2, 384], F32, tag="s2ps")
        for i in range(3):
            nc.tensor.matmul(s2ps[:, 1:384], occ[i], Sst[i][:, 1:384],
                             start=(i == 0), stop=(i == 2))
        s2 = tabs.tile([2, 384], F32, tag="s2")
        nc.vector.memset(s2[:, 0:1], 0.0)
        nc.vector.tensor_copy(out=s2[:, 1:384], in_=s2ps[:, 1:384])

        # per-partition inverse scales: wv = 2/768 (p0 of chunk0 -> 1/768)
        wv = consts.tile([P, 1], F32, tag="wv")
        nc.vector.memset(wv, 2.0 / D)
        wv0 = consts.tile([P, 1], F32, tag="wv0")
        nc.vector.memset(wv0, 2.0 / D)
        nc.vector.memset(wv0[0:1, :], 1.0 / D)

        FPt, GPt = [], []
        for c in range(NCH):
            t = consts.tile([P, D], mm_dtype, tag=f"fp{c}", name=f"fp{c}")
            if c <= 2:
                nc.scalar.copy(out=t[:, 0:385], in_=Cst[c])
                nc.gpsimd.memset(t[:, 385:768].bitcast(F32), 0.0)
            elif c == 3:
                nc.scalar.mul(out=t[:, 385:768], in_=Sst[0][:, 1:384],
                              mul=-1.0)  # row 0 of S is zero
                nc.gpsimd.memset(t[:, 0:385].bitcast(F32), 0.0)
                nc.scalar.copy(out=t[0:1, 0:385], in_=c384)
            else:
                nc.gpsimd.memset(t[:, 0:385].bitcast(F32), 0.0)
                nc.scalar.mul(out=t[:, 385:768], in_=Sst[c - 3][:, 1:384],
                              mul=-1.0)
            FPt.append(t)
        for c in range(NCH):
            t = consts.tile([P, 770], mm_dtype, tag=f"gp{c}", name=f"gp{c}")
            if c <= 2:
                nc.scalar.mul(out=t[:, 0:385], in_=Cst[c],
                              mul=(wv0 if c == 0 else wv))
                nc.gpsimd.memset(t[:, 385:770].bitcast(F32), 0.0)
            elif c == 3:
                nc.scalar.mul(out=t[:, 386:769], in_=Sst[0][:, 1:384],
                              mul=2.0 / D)  # row 0 of S is zero
                nc.gpsimd.memset(t[:, 0:386].bitcast(F32), 0.0)
                nc.gpsimd.memset(t[:, 769:770].bitcast(F32), 0.0)
                nc.scalar.mul(out=t[0:1, 0:385], in_=c384, mul=1.0 / D)
            else:
                nc.gpsimd.memset(t[:, 0:386].bitcast(F32), 0.0)
                nc.scalar.mul(out=t[:, 386:769], in_=Sst[c - 3][:, 1:384],
                              mul=2.0 / D)
                nc.gpsimd.memset(t[:, 769:770].bitcast(F32), 0.0)
            GPt.append(t)

        # ---- positions ----
        posT = tabs.tile([2, N], I32, tag="posT")
        nc.sync.dma_start(out=posT, in_=positions.rearrange("n k -> k n"))
        posTf = tabs.tile([2, N], F32, tag="posTf")
        nc.vector.tensor_scalar_mul(posTf, posT, -2.0)

        # ---- theta -> cos/sin tables (3 chunks of (128, N)) ----
        cT, sT = [], []
        for j in range(3):
            thps = psum.tile([P, N], F32, tag="thps")
            for h in range(2):
                nc.tensor.matmul(thps[:, h * 512:(h + 1) * 512],
                                 s2[:, j * P:(j + 1) * P],
                                 posTf[:, h * 512:(h + 1) * 512],
                                 start=True, stop=True)
            sj = [tabs.tile([P, 512], F32, tag=f"sT{j}_{hh}",
                            name=f"sT{j}_{hh}") for hh in range(2)]
            cj = [tabs.tile([P, 512], F32, tag=f"cT{j}_{hh}",
                            name=f"cT{j}_{hh}") for hh in range(2)]
            for hh in range(2):
                hs = slice(hh * 512, (hh + 1) * 512)
                te = stage.tile([P, 512], F32, tag="te")
                nc.scalar.copy(out=te, in_=thps[:, hs])
                t1 = stage.tile([P, 512], F32, tag="pt")
                r1 = stage.tile([P, 512], I32, tag="pr")
                u1 = stage.tile([P, 512], F32, tag="pu")
                red = stage.tile([P, 512], F32, tag="pred")
                nc.vector.tensor_scalar_mul(t1, te, 1.0 / TWOPI)
                nc.vector.tensor_copy(out=r1, in_=t1)
                nc.vector.tensor_scalar_mul(u1, r1, -TWOPI)
                nc.vector.tensor_add(red, te, u1)
                nc.scalar.activation(out=sj[hh], in_=red,
                                     func=mybir.ActivationFunctionType.Sin)
                t2 = stage.tile([P, 512], F32, tag="qt")
                r2 = stage.tile([P, 512], I32, tag="qr")
                u2 = stage.tile([P, 512], F32, tag="qu")
                red2 = stage.tile([P, 512], F32, tag="qred")
                nc.gpsimd.tensor_scalar(t2, te, 1.0 / TWOPI, 0.25,
                                        op0=mybir.AluOpType.mult,
                                        op1=mybir.AluOpType.add)
                nc.vector.tensor_copy(out=r2, in_=t2)
                nc.gpsimd.tensor_scalar(u2, r2, -TWOPI, math.pi / 2,
                                        op0=mybir.AluOpType.mult,
                                        op1=mybir.AluOpType.add)
                nc.gpsimd.tensor_add(red2, te, u2)
                nc.scalar.activation(out=cj[hh], in_=red2,
                                     func=mybir.ActivationFunctionType.Sin)
            sT.append(sj)
            cT.append(cj)
        hp.__exit__(None, None, None)
        ps0.__exit__(None, None, None)

        # ---- main loop ----
        pst_pool = ctx.enter_context(tc.tile_pool(name="pst", bufs=1,
                                                  space="PSUM"))
        psf = ctx.enter_context(tc.tile_pool(name="psf", bufs=5, space="PSUM"))
        psi = ctx.enter_context(tc.tile_pool(name="psi", bufs=1, space="PSUM"))
        for rep in range(reps):
          for b in range(BS):
            for h in range(2):
                n0 = h * ROWTILE
                # load 4 row groups; even/odd fold on Pool/DVE
                eog = []
                for g in range(NG):
                    t = xio.tile([P, D], F32, tag=f"x{g % 2}")
                    nc.sync.dma_start(
                        out=t, in_=x[b, n0 + g * P:n0 + (g + 1) * P, :])
                    eo = xio.tile([P, D], F32, tag=f"eo{g}")
                    nc.gpsimd.tensor_add(eo[:, 1:384], t[:, 1:384],
                                         t[:, 767:384:-1])
                    nc.gpsimd.tensor_sub(eo[:, 385:768], t[:, 1:384],
                                         t[:, 767:384:-1])
                    nc.vector.tensor_copy(out=eo[:, 0:385:384],
                                          in_=t[:, 0:385:384])
                    eog.append(eo)
                # transpose eo to (d', rows): 6 chunks of (128, 512), fp32r
                XT = []
                for c in range(NCH):
                    pst = pst_pool.tile([P, ROWTILE], F32, tag="pst")
                    for g in range(NG):
                        nc.tensor.transpose(pst[:, g * P:(g + 1) * P],
                                            eog[g][:, c * P:(c + 1) * P],
                                            ident)
                    xt = work.tile([P, ROWTILE], mm_dtype, tag=f"xt{c}")
                    nc.scalar.copy(out=xt, in_=pst)
                    XT.append(xt)
                # forward (block-sparse) + rotation per pair (j, 3+j)
                RI = [None] * NCH
                for j in range(3):
                    pR = psf.tile([P, ROWTILE], F32, tag="psf")
                    pI = psf.tile([P, ROWTILE], F32, tag="psf")
                    kR = FWD_BLOCKS[j]
                    for i, c in enumerate(kR):
                        nc.tensor.matmul(pR, FPt[c][:, j * P:(j + 1) * P],
                                         XT[c], start=(i == 0),
                                         stop=(i == len(kR) - 1))
                    kI = FWD_BLOCKS[3 + j]
                    for i, c in enumerate(kI):
                        nc.tensor.matmul(pI,
                                         FPt[c][:, (3 + j) * P:(4 + j) * P],
                                         XT[c], start=(i == 0),
                                         stop=(i == len(kI) - 1))
                    cs = cT[j][h]
                    sn = sT[j][h]
                    t1 = work.tile([P, ROWTILE], F32, tag="rta")
                    t2 = work.tile([P, ROWTILE], F32, tag="rtb")
                    t3 = work.tile([P, ROWTILE], F32, tag="rtc")
                    t4 = work.tile([P, ROWTILE], F32, tag="rtd")
                    nc.vector.tensor_mul(t1, pR, cs)
                    nc.vector.tensor_mul(t3, pR, sn)
                    nc.vector.tensor_mul(t2, pI, sn)
                    nc.vector.tensor_mul(t4, pI, cs)
                    rp = work.tile([P, ROWTILE], mm_dtype, tag=f"ri{j}")
                    ip = work.tile([P, ROWTILE], mm_dtype, tag=f"ri{3 + j}")
                    nc.gpsimd.tensor_sub(rp, t1, t2)
                    nc.gpsimd.tensor_add(ip, t3, t4)
                    RI[j] = rp
                    RI[3 + j] = ip
                # inverse (folded): u (385) and v (383) psum, un-fold to osb
                for g in range(NG):
                    pa = psi.tile([P, 386], F32, tag="pa")
                    pb = psi.tile([P, 384], F32, tag="pb")
                    gs = slice(g * P, (g + 1) * P)
                    for i, c in enumerate((0, 1, 2, 3)):
                        nc.tensor.matmul(pa, RI[c][:, gs], GPt[c][:, 0:386],
                                         start=(i == 0), stop=(i == 3))
                    for i, c in enumerate((3, 4, 5)):
                        nc.tensor.matmul(pb, RI[c][:, gs], GPt[c][:, 386:770],
                                         start=(i == 0), stop=(i == 2))
                    vb = work.tile([P, 384], F32, tag="rta")
                    ua = work.tile([P, 386], F32, tag="rtb")
                    nc.scalar.copy(out=vb, in_=pb)
                    nc.scalar.copy(out=ua, in_=pa)
                    osb = xio.tile([P, D], F32, tag=f"eo{g}")
                    nc.gpsimd.tensor_sub(osb[:, 1:384], ua[:, 1:384],
                                         vb[:, 0:383])
                    nc.gpsimd.tensor_add(osb[:, 385:768], ua[:, 383:0:-1],
                                         vb[:, 382::-1])
                    nc.vector.tensor_copy(out=osb[:, 0:385:384],
                                          in_=ua[:, 0:385:384])
                    nc.sync.dma_start(
                        out=out[b, n0 + g * P:n0 + (g + 1) * P, :], in_=osb)
    nc.finalize()
    return nc


_NC_CACHE = {}


def kernel(x, circ, positions):
    x = np.ascontiguousarray(x, dtype=np.float32)
    circ = np.ascontiguousarray(circ, dtype=np.float32)
    positions = np.ascontiguousarray(positions, dtype=np.int32)
    if "nc" not in _NC_CACHE:
        _NC_CACHE["nc"] = build_kernel()
    nc = _NC_CACHE["nc"]
    FP, GP = _dft_matrices()
    in_maps = []
    for core in range(NCORES):
        in_maps.append({
            "x": x[core * BS:(core + 1) * BS],
            "circ": circ,
            "positions": positions,
            "cs_c": FP,
            "ss_c": GP,
        })
    res = bass_utils.run_bass_kernel_spmd(nc, in_maps,
                                          core_ids=list(range(NCORES)))
    out = np.concatenate([res.results[c]["out"] for c in range(NCORES)],
                         axis=0)
    return out


if __name__ == "__main__":
    rng = np.random.default_rng(0)
    x = rng.standard_normal((B, N, D)).astype(np.float32)
    circ = (rng.standard_normal((2, D)) * 0.01).astype(np.float32)
    positions = rng.integers(0, 32, (N, 2)).astype(np.int32)
    out = kernel(x=x, circ=circ, positions=positions)
    print("out", out.shape, out.dtype)
